# revision 35
# baseline (speedup 1.0000x reference)
"""KAN layer on 8 Trainium2 NeuronCores (Bass/Tile).

Computes out = x @ base_weight.T + silu(x) @ spline_weight.sum(-1).T
for x:[8192,1024] f32, base_weight:[1024,1024] f32,
spline_weight:[1024,1024,8] f32 -> out:[8192,1024] f32.

Strategy (self-contained, hardcoded for these shapes):
  * 2D shard over the 8 cores: batch split R=2, out-features split C=4.
    Core (r, c) computes out[4096r:4096(r+1), 256c:256(c+1)].
  * Host prep is layout + dtype narrowing only (transpose/reshape/slice
    plus f32->bf16 / f32->fp8e4m3 rounding -- the same rounding the
    device DVE would apply; all KAN math runs on device).  Narrow
    transfers cut per-core HBM traffic 28.3MB -> ~14.9MB, which was the
    baseline bottleneck (DMA busy ~79us at the ~358GB/s/core limit).
  * On-device per core: silu on ScalarE, spline g-axis reduce as DVE
    tree adds, x pair-cast bf16->fp8 on DVE.  Matmuls run
    weight-stationary with the batch stream as the moving operand
    (N=512 columns/matmul): the spline term (dominant magnitude) in
    bf16, the base term in fp8e4m3 DoubleRow (2x PE rate; base is only
    ~17% of output magnitude so the fp8 rounding contributes <1%
    relative error).  Both accumulate f32 into the same PSUM banks
    (K=2048 fused), 8 full banks, two batch passes of 2048.
  * Output is written bf16 and upcast to f32 on gather.
"""
import sys

for _p in ("/opt/trn_rl_repo",):
    if _p not in sys.path:
        sys.path.insert(0, _p)

import ml_dtypes
import numpy as np

import concourse.bass as bass  # noqa: F401  (bass must import before mybir use)
import concourse.mybir as mybir
import concourse.tile as tile
from concourse import bacc
from concourse.bass_utils import run_bass_kernel_spmd

P = 128
IN_F = 1024
G = 8
N_CORES = 8
R_SPLIT = 2
C_SPLIT = 4
B_LOC = 8192 // R_SPLIT      # 4096 batch rows per core
O_LOC = 1024 // C_SPLIT      # 256 out features per core
KT = IN_F // P               # 8 k-tiles over in_features
KS = KT // 2                 # 4 fp8 DoubleRow super-tiles (K=256 each)
N_MM = 512                   # moving (batch) columns per matmul = 1 PSUM bank
N_PASS = 2                   # batch passes of 2048
B_PASS = B_LOC // N_PASS     # 2048
J_SUB = B_PASS // N_MM       # 4 banks per o-tile per pass
O_TILES = O_LOC // P         # 2

F32 = mybir.dt.float32
BF16 = mybir.dt.bfloat16
F8E4 = mybir.dt.float8e4
AF = mybir.ActivationFunctionType
DR = mybir.MatmulPerfMode.DoubleRow

_compiled = None


def _build_kernel():
    nc = bacc.Bacc(None, target_bir_lowering=False, num_devices=N_CORES)
    xt = nc.dram_tensor("xt", [KS, N_PASS, P, 2, B_PASS], BF16,
                        kind="ExternalInput")
    st = nc.dram_tensor("st", [KS, P, 2, G, O_LOC], BF16, kind="ExternalInput")
    bt = nc.dram_tensor("bt", [P, KS, 2, O_LOC], F8E4, kind="ExternalInput")
    out = nc.dram_tensor("out", [O_TILES, P, B_LOC], BF16,
                         kind="ExternalOutput")

    with tile.TileContext(nc) as tc:
        with (
            tc.tile_pool(name="wconst", bufs=1) as wconst,
            tc.tile_pool(name="wstage", bufs=4) as wstage,
            tc.tile_pool(name="gstage", bufs=1) as gstage,
            tc.tile_pool(name="xfring", bufs=8) as xfring,
            tc.tile_pool(name="slring", bufs=6) as slring,
            tc.tile_pool(name="xqring", bufs=5) as xqring,
            tc.tile_pool(name="psum", bufs=1, space="PSUM") as psum,
            tc.tile_pool(name="opool", bufs=2) as opool,
        ):
            # ---- PE warm-up: junk matmuls so HAM is at 8/8 for real work
            # (zeroed tile, result never read; targets the bank whose first
            # real matmul comes latest).
            junk = wconst.tile([P, N_MM], BF16, name="junk")
            nc.vector.memset(junk[:], 0)
            jp = psum.tile([P, N_MM], F32, name="acc1_3")
            for _ in range(26):
                nc.tensor.matmul(jp[:], junk[:, :P], junk[:],
                                 start=True, stop=True,
                                 skip_group_check=True)

            wb = wconst.tile([P, KS, 2, O_LOC], F8E4, name="wb")
            ws = [None] * KS
            xf = [[None] * N_PASS for _ in range(KS)]
            slp = [[None] * N_PASS for _ in range(KS)]
            xq = [[None] * N_PASS for _ in range(KS)]

            def dma_x(c, h, eng):
                t = xfring.tile([P, 2, B_PASS], BF16, name="xf", tag="xf")
                eng.dma_start(t[:], xt[c, h])
                xf[c][h] = t

            def silu_x(c, h):
                sl = slring.tile([P, 2, B_PASS], BF16, name="slp", tag="slp")
                for i in range(2):
                    nc.scalar.activation(sl[:, i], xf[c][h][:, i], AF.Silu)
                slp[c][h] = sl

            def cast_x(c, h):
                xqc = xqring.tile([P, 2, B_PASS], F8E4, name="xq", tag="xq")
                nc.vector.tensor_copy(xqc[:], xf[c][h][:])
                xq[c][h] = xqc

            stgs = [None] * KS

            def dma_st(c, eng):
                stg = wstage.tile([P, 2, G, O_LOC], BF16, name="stg",
                                  tag="stg")
                eng.dma_start(stg[:], st[c])
                stgs[c] = stg

            def gsum_st(c):
                stg = stgs[c]
                t1 = gstage.tile([P, 2, 4, O_LOC], BF16, name="gs1")
                nc.vector.tensor_add(t1[:], stg[:, :, 0:4], stg[:, :, 4:8])
                t2 = gstage.tile([P, 2, 2, O_LOC], BF16, name="gs2")
                nc.vector.tensor_add(t2[:], t1[:, :, 0:2], t1[:, :, 2:4])
                wsc = wconst.tile([P, 2, O_LOC], BF16, name=f"ws{c}")
                nc.vector.tensor_add(wsc[:], t2[:, :, 0], t2[:, :, 1])
                ws[c] = wsc

            # ---- all input DMAs up front, need order, alternating between
            # the two HWDGE rings (sync / scalar) so receipts overlap ----
            nc.sync.dma_start(wb[:], bt[:])
            dma_x(0, 0, nc.scalar)
            dma_st(0, nc.sync)
            dma_x(1, 0, nc.scalar)
            dma_st(1, nc.sync)
            dma_x(2, 0, nc.scalar)
            dma_st(2, nc.sync)
            dma_x(3, 0, nc.scalar)
            dma_st(3, nc.sync)
            dma_x(0, 1, nc.scalar)
            dma_x(1, 1, nc.sync)
            dma_x(2, 1, nc.scalar)
            dma_x(3, 1, nc.sync)
            # preps in consumption order (per-engine queues are FIFO)
            for c in range(KS):
                silu_x(c, 0)
                cast_x(c, 0)
                gsum_st(c)

            # block sequences: pass 0 opens with fp8 s=0 (cast ready first);
            # pass 1 opens with spline k0 (silu ready before the post-drain
            # cast); the silu-latest k-tiles close both passes.
            SEQS = [
                [("s", 0), ("k", 0), ("k", 1), ("k", 2), ("k", 3),
                 ("s", 1), ("k", 4), ("k", 5), ("s", 2), ("s", 3),
                 ("k", 6), ("k", 7)],
                [("k", 0), ("k", 1), ("s", 0), ("k", 2), ("k", 3),
                 ("s", 1), ("k", 4), ("k", 5), ("s", 2), ("s", 3),
                 ("k", 6), ("k", 7)],
            ]

            def mm_all(p, accs, typ, i, start, stop):
                """One stationary-weight block: all 8 banks x both o-tiles."""
                for o in range(O_TILES):
                    osl = slice(P * o, P * (o + 1))
                    for j in range(J_SUB):
                        bsl = slice(N_MM * j, N_MM * (j + 1))
                        a = accs[J_SUB * o + j][:]
                        if typ == "s":
                            nc.tensor.matmul(
                                a, wb[:, i, :, osl], xq[i][p][:, :, bsl],
                                start=start, stop=stop, perf_mode=DR)
                        else:
                            nc.tensor.matmul(
                                a, ws[i // 2][:, i % 2, osl],
                                slp[i // 2][p][:, i % 2, bsl],
                                start=start, stop=stop)

            def mm_pass(p):
                accs = [psum.tile([P, N_MM], F32, name=f"acc{o}_{j}")
                        for o in range(O_TILES) for j in range(J_SUB)]
                seq = SEQS[p]
                for idx, (typ, i) in enumerate(seq):
                    mm_all(p, accs, typ, i,
                           start=(idx == 0), stop=(idx == len(seq) - 1))
                for o in range(O_TILES):
                    ot = opool.tile([P, J_SUB, N_MM], BF16, name="ot",
                                    tag=f"ot{o}")
                    for j in range(J_SUB):
                        if p == N_PASS - 1 and o == 1:
                            nc.scalar.activation(ot[:, j],
                                                 accs[J_SUB * o + j][:],
                                                 AF.Copy)
                        else:
                            nc.vector.tensor_copy(ot[:, j],
                                                  accs[J_SUB * o + j][:])
                    nc.sync.dma_start(
                        out[o, :, B_PASS * p:B_PASS * (p + 1)], ot[:])

            mm_pass(0)
            for c in range(KS):
                silu_x(c, 1)
                cast_x(c, 1)
            mm_pass(1)
    nc.compile()
    return nc


def _get_compiled():
    global _compiled
    if _compiled is None:
        _compiled = _build_kernel()
    return _compiled


def _shard_inputs(x, base_weight, spline_weight):
    """Full inputs -> 8 per-core in_maps (layout + dtype narrowing)."""
    x = np.ascontiguousarray(np.asarray(x, dtype=np.float32))
    base_weight = np.ascontiguousarray(np.asarray(base_weight, dtype=np.float32))
    spline_weight = np.ascontiguousarray(np.asarray(spline_weight, dtype=np.float32))

    # x.T -> [KT, P, 8192] bf16 (per-core batch slice + half split below)
    xt_full = np.ascontiguousarray(x.T.reshape(KT, P, 8192)
                                   .astype(ml_dtypes.bfloat16))
    # base_weight.T [in, out] -> [P, KS, 2, out] fp8 (per-core out slice below)
    btf = (base_weight.T.reshape(KS, 2, P, 1024).transpose(2, 0, 1, 3)
           .astype(ml_dtypes.float8_e4m3))
    # spline [out, in, g] -> [in, g, out] -> [KS, P, 2, G, out] bf16
    stf = (spline_weight.transpose(1, 2, 0).reshape(KS, 2, P, G, 1024)
           .transpose(0, 2, 1, 3, 4).astype(ml_dtypes.bfloat16))

    in_maps = []
    for core in range(N_CORES):
        r, c = divmod(core, C_SPLIT)
        osl = slice(O_LOC * c, O_LOC * (c + 1))
        xs = (xt_full[:, :, B_LOC * r:B_LOC * (r + 1)]
              .reshape(KS, 2, P, N_PASS, B_PASS).transpose(0, 3, 2, 1, 4))
        in_maps.append({
            "xt": np.ascontiguousarray(xs),
            "st": np.ascontiguousarray(stf[:, :, :, :, osl]),
            "bt": np.ascontiguousarray(btf[:, :, :, osl]),
        })
    return in_maps


def _gather_output(results):
    out = np.empty((8192, 1024), dtype=np.float32)
    for core in range(N_CORES):
        r, c = divmod(core, C_SPLIT)
        oc = results[core]["out"].astype(np.float32)   # [2 o, 128 p, 4096 b]
        oc = oc.reshape(O_LOC, B_LOC).T                # [4096 b, 256 o]
        out[B_LOC * r:B_LOC * (r + 1), O_LOC * c:O_LOC * (c + 1)] = oc
    return out


def run(trace=False, **inputs):
    """Run on the 8 NeuronCores; returns (out, BassKernelResults)."""
    nc = _get_compiled()
    in_maps = _shard_inputs(**inputs)
    res = run_bass_kernel_spmd(
        nc, in_maps, core_ids=list(range(N_CORES)), trace=trace)
    return _gather_output(res.results), res


def kernel(**inputs) -> np.ndarray:
    out, _ = run(trace=False, **inputs)
    return out


# revision 36
# speedup vs baseline: 1.1865x; 1.1865x over previous
"""KAN layer on 8 Trainium2 NeuronCores (Bass/Tile).

Computes out = x @ base_weight.T + silu(x) @ spline_weight.sum(-1).T
for x:[8192,1024] f32, base_weight:[1024,1024] f32,
spline_weight:[1024,1024,8] f32 -> out:[8192,1024] f32.

Strategy (self-contained, hardcoded for these shapes):
  * 2D shard over the 8 cores: batch split R=2, out-features split C=4.
    Core (r, c) computes out[4096r:4096(r+1), 256c:256(c+1)].
  * Host prep is layout + dtype narrowing only (transpose/reshape/slice
    plus f32->bf16 / f32->fp8e4m3 rounding -- the same rounding the
    device DVE would apply; all KAN math runs on device).  Narrow
    transfers cut per-core HBM traffic 28.3MB -> ~14.9MB, which was the
    baseline bottleneck (DMA busy ~79us at the ~358GB/s/core limit).
  * On-device per core: silu on ScalarE, spline g-axis reduce as DVE
    tree adds, x pair-cast bf16->fp8 on DVE.  Matmuls run
    weight-stationary with the batch stream as the moving operand
    (N=512 columns/matmul): the spline term (dominant magnitude) in
    bf16, the base term in fp8e4m3 DoubleRow (2x PE rate; base is only
    ~17% of output magnitude so the fp8 rounding contributes <1%
    relative error).  Both accumulate f32 into the same PSUM banks
    (K=2048 fused), 8 full banks, two batch passes of 2048.
  * Output is written bf16 and upcast to f32 on gather.
"""
import sys

for _p in ("/opt/trn_rl_repo",):
    if _p not in sys.path:
        sys.path.insert(0, _p)

import ml_dtypes
import numpy as np

import concourse.bass as bass  # noqa: F401  (bass must import before mybir use)
import concourse.mybir as mybir
import concourse.tile as tile
from concourse import bacc
from concourse.bass_utils import run_bass_kernel_spmd

P = 128
IN_F = 1024
G = 8
N_CORES = 8
R_SPLIT = 2
C_SPLIT = 4
B_LOC = 8192 // R_SPLIT      # 4096 batch rows per core
O_LOC = 1024 // C_SPLIT      # 256 out features per core
KT = IN_F // P               # 8 k-tiles over in_features
KS = KT // 2                 # 4 fp8 DoubleRow super-tiles (K=256 each)
N_MM = 512                   # moving (batch) columns per matmul = 1 PSUM bank
N_PASS = 2                   # batch passes of 2048
B_PASS = B_LOC // N_PASS     # 2048
J_SUB = B_PASS // N_MM       # 4 banks per o-tile per pass
O_TILES = O_LOC // P         # 2

F32 = mybir.dt.float32
BF16 = mybir.dt.bfloat16
F8E4 = mybir.dt.float8e4
AF = mybir.ActivationFunctionType
DR = mybir.MatmulPerfMode.DoubleRow

_compiled = None


def _build_kernel():
    nc = bacc.Bacc(None, target_bir_lowering=False, num_devices=N_CORES)
    xt = nc.dram_tensor("xt", [KS, N_PASS, P, 2, B_PASS], BF16,
                        kind="ExternalInput")
    st = nc.dram_tensor("st", [KS, P, 2, G, O_LOC], BF16, kind="ExternalInput")
    bt = nc.dram_tensor("bt", [P, KS, 2, O_LOC], F8E4, kind="ExternalInput")
    out = nc.dram_tensor("out", [O_TILES, P, B_LOC], BF16,
                         kind="ExternalOutput")

    with tile.TileContext(nc) as tc:
        with (
            tc.tile_pool(name="wconst", bufs=1) as wconst,
            tc.tile_pool(name="wstage", bufs=4) as wstage,
            tc.tile_pool(name="gstage", bufs=1) as gstage,
            tc.tile_pool(name="xfring", bufs=8) as xfring,
            tc.tile_pool(name="slring", bufs=6) as slring,
            tc.tile_pool(name="xqring", bufs=5) as xqring,
            tc.tile_pool(name="psum", bufs=1, space="PSUM") as psum,
            tc.tile_pool(name="opool", bufs=2) as opool,
        ):
            # ---- PE warm-up: junk matmuls so HAM is at 8/8 for real work
            # (zeroed tile, result never read; targets the bank whose first
            # real matmul comes latest).
            junk = wconst.tile([P, N_MM], BF16, name="junk")
            nc.vector.memset(junk[:], 0)
            jp = psum.tile([P, N_MM], F32, name="acc1_3")
            for _ in range(26):
                nc.tensor.matmul(jp[:], junk[:, :P], junk[:],
                                 start=True, stop=True,
                                 skip_group_check=True)

            wb = wconst.tile([P, KS, 2, O_LOC], F8E4, name="wb")
            ws = [None] * KS
            xf = [[None] * N_PASS for _ in range(KS)]
            slp = [[None] * N_PASS for _ in range(KS)]
            xq = [[None] * N_PASS for _ in range(KS)]

            def dma_x(c, h, eng):
                t = xfring.tile([P, 2, B_PASS], BF16, name="xf", tag="xf")
                eng.dma_start(t[:], xt[c, h])
                xf[c][h] = t

            def silu_x(c, h):
                sl = slring.tile([P, 2, B_PASS], BF16, name="slp", tag="slp")
                for i in range(2):
                    nc.scalar.activation(sl[:, i], xf[c][h][:, i], AF.Silu)
                slp[c][h] = sl

            def cast_x(c, h):
                xqc = xqring.tile([P, 2, B_PASS], F8E4, name="xq", tag="xq")
                nc.vector.tensor_copy(xqc[:], xf[c][h][:])
                xq[c][h] = xqc

            stgs = [None] * KS

            def dma_st(c, eng):
                stg = wstage.tile([P, 2, G, O_LOC], BF16, name="stg",
                                  tag="stg")
                eng.dma_start(stg[:], st[c])
                stgs[c] = stg

            def gsum_st(c):
                stg = stgs[c]
                t1 = gstage.tile([P, 2, 4, O_LOC], BF16, name="gs1")
                nc.vector.tensor_add(t1[:], stg[:, :, 0:4], stg[:, :, 4:8])
                t2 = gstage.tile([P, 2, 2, O_LOC], BF16, name="gs2")
                nc.vector.tensor_add(t2[:], t1[:, :, 0:2], t1[:, :, 2:4])
                wsc = wconst.tile([P, 2, O_LOC], BF16, name=f"ws{c}")
                nc.vector.tensor_add(wsc[:], t2[:, :, 0], t2[:, :, 1])
                ws[c] = wsc

            # ---- all input DMAs up front on one HWDGE ring, need order ----
            nc.sync.dma_start(wb[:], bt[:])
            dma_x(0, 0, nc.sync)
            dma_st(0, nc.sync)
            dma_x(1, 0, nc.sync)
            dma_st(1, nc.sync)
            dma_x(2, 0, nc.sync)
            dma_st(2, nc.sync)
            dma_x(3, 0, nc.sync)
            dma_st(3, nc.sync)
            dma_x(0, 1, nc.sync)
            dma_x(1, 1, nc.sync)
            dma_x(2, 1, nc.sync)
            dma_x(3, 1, nc.sync)
            # preps in consumption order (per-engine queues are FIFO)
            for c in range(KS):
                silu_x(c, 0)
                cast_x(c, 0)
                gsum_st(c)

            # block sequences: pass 0 opens with fp8 s=0 (cast ready first);
            # pass 1 opens with spline k0 (silu ready before the post-drain
            # cast); the silu-latest k-tiles close both passes.
            SEQS = [
                [("s", 0), ("k", 0), ("k", 1), ("k", 2), ("k", 3),
                 ("s", 1), ("k", 4), ("k", 5), ("s", 2), ("s", 3),
                 ("k", 6), ("k", 7)],
                [("k", 0), ("k", 1), ("s", 0), ("k", 2), ("k", 3),
                 ("s", 1), ("k", 4), ("k", 5), ("s", 2), ("s", 3),
                 ("k", 6), ("k", 7)],
            ]

            def mm_all(p, accs, typ, i, start, stop):
                """One stationary-weight block: all 8 banks x both o-tiles."""
                for o in range(O_TILES):
                    osl = slice(P * o, P * (o + 1))
                    for j in range(J_SUB):
                        bsl = slice(N_MM * j, N_MM * (j + 1))
                        a = accs[J_SUB * o + j][:]
                        if typ == "s":
                            nc.tensor.matmul(
                                a, wb[:, i, :, osl], xq[i][p][:, :, bsl],
                                start=start, stop=stop, perf_mode=DR)
                        else:
                            nc.tensor.matmul(
                                a, ws[i // 2][:, i % 2, osl],
                                slp[i // 2][p][:, i % 2, bsl],
                                start=start, stop=stop)

            def mm_pass(p):
                accs = [psum.tile([P, N_MM], F32, name=f"acc{o}_{j}")
                        for o in range(O_TILES) for j in range(J_SUB)]
                seq = SEQS[p]
                for idx, (typ, i) in enumerate(seq):
                    mm_all(p, accs, typ, i,
                           start=(idx == 0), stop=(idx == len(seq) - 1))
                for o in range(O_TILES):
                    ot = opool.tile([P, J_SUB, N_MM], BF16, name="ot",
                                    tag=f"ot{o}")
                    for j in range(J_SUB):
                        if p == N_PASS - 1 and o == 1:
                            nc.scalar.activation(ot[:, j],
                                                 accs[J_SUB * o + j][:],
                                                 AF.Copy)
                        else:
                            nc.vector.tensor_copy(ot[:, j],
                                                  accs[J_SUB * o + j][:])
                    nc.sync.dma_start(
                        out[o, :, B_PASS * p:B_PASS * (p + 1)], ot[:])

            mm_pass(0)
            for c in range(KS):
                silu_x(c, 1)
                cast_x(c, 1)
            mm_pass(1)
    nc.compile()
    return nc


def _get_compiled():
    global _compiled
    if _compiled is None:
        _compiled = _build_kernel()
    return _compiled


def _shard_inputs(x, base_weight, spline_weight):
    """Full inputs -> 8 per-core in_maps (layout + dtype narrowing)."""
    x = np.ascontiguousarray(np.asarray(x, dtype=np.float32))
    base_weight = np.ascontiguousarray(np.asarray(base_weight, dtype=np.float32))
    spline_weight = np.ascontiguousarray(np.asarray(spline_weight, dtype=np.float32))

    # x.T -> [KT, P, 8192] bf16 (per-core batch slice + half split below)
    xt_full = np.ascontiguousarray(x.T.reshape(KT, P, 8192)
                                   .astype(ml_dtypes.bfloat16))
    # base_weight.T [in, out] -> [P, KS, 2, out] fp8 (per-core out slice below)
    btf = (base_weight.T.reshape(KS, 2, P, 1024).transpose(2, 0, 1, 3)
           .astype(ml_dtypes.float8_e4m3))
    # spline [out, in, g] -> [in, g, out] -> [KS, P, 2, G, out] bf16
    stf = (spline_weight.transpose(1, 2, 0).reshape(KS, 2, P, G, 1024)
           .transpose(0, 2, 1, 3, 4).astype(ml_dtypes.bfloat16))

    in_maps = []
    for core in range(N_CORES):
        r, c = divmod(core, C_SPLIT)
        osl = slice(O_LOC * c, O_LOC * (c + 1))
        xs = (xt_full[:, :, B_LOC * r:B_LOC * (r + 1)]
              .reshape(KS, 2, P, N_PASS, B_PASS).transpose(0, 3, 2, 1, 4))
        in_maps.append({
            "xt": np.ascontiguousarray(xs),
            "st": np.ascontiguousarray(stf[:, :, :, :, osl]),
            "bt": np.ascontiguousarray(btf[:, :, :, osl]),
        })
    return in_maps


def _gather_output(results):
    out = np.empty((8192, 1024), dtype=np.float32)
    for core in range(N_CORES):
        r, c = divmod(core, C_SPLIT)
        oc = results[core]["out"].astype(np.float32)   # [2 o, 128 p, 4096 b]
        oc = oc.reshape(O_LOC, B_LOC).T                # [4096 b, 256 o]
        out[B_LOC * r:B_LOC * (r + 1), O_LOC * c:O_LOC * (c + 1)] = oc
    return out


def run(trace=False, **inputs):
    """Run on the 8 NeuronCores; returns (out, BassKernelResults)."""
    nc = _get_compiled()
    in_maps = _shard_inputs(**inputs)
    res = run_bass_kernel_spmd(
        nc, in_maps, core_ids=list(range(N_CORES)), trace=trace)
    return _gather_output(res.results), res


def kernel(**inputs) -> np.ndarray:
    out, _ = run(trace=False, **inputs)
    return out
